# revision 25
# baseline (speedup 1.0000x reference)
"""Trainium2 Bass kernel for MinibatchDiscrimination2d.

Full computation:
  x (32,128,64,64) --conv s4--> x_r (32,3,16,16)
  M = x_r @ T  -> (32, 8192, 16)
  dist[b1,b2,d] = sum_f |M[b1,d,f]-M[b2,d,f]|
  out[b,d] = sum_b2 exp(-dist) - 1 -> (32,32,16,16)
  out_a = deconv s4 (32,32,64,64); return concat([x, out_a], ch)

Sharding over 8 cores: split the t*t=256 output spatial positions of the
D_OUT axis into 8 row-bands (2 of 16 t-rows per core). Each core gets a
(768, 1024, 16) slice of T (fp8 e4m3, x128 scale), computes M/dist/out
for its band for ALL 32 samples, and deconvs its band into 8 of the 64
output rows. The conv runs data-parallel over B (4 samples/core, fp8
DoubleRow) followed by an AllGather of the tiny x_r. x passes through on
the host (concat identity part).

fp8: T scaled x128, xr scaled x2 -> M psum = 256*M, descaled on the
PSUM->SBUF copy. M-matmul and conv use DoubleRow (2 contraction rows
per partition).

Per-core d index:  s = (r*16 + j)*32 + ch   (r in 0..1, j in 0..15, ch in 0..31)
dgroup g = s // 128; partition p = s % 128 = (rj%4)*32 + ch.
"""

import numpy as np
import ml_dtypes

N_CORES = 8
B, IN_FLT, N = 32, 128, 64
K = 4
T_SP = 16
OC = 32
F = 16
D_IN = 768
BC = B // N_CORES          # 4 samples per core (conv data-parallel)
DSH = 1024                 # d per core
NG = DSH // 128            # 8 dgroups
KP = 3                     # contraction pairs (768 = 3 * 256) for DoubleRow
T_SCALE = 128.0
XR_SCALE = 2.0
M_DESCALE = 1.0 / (T_SCALE * XR_SCALE)

_CACHE = {}


def _build_nc():
    import concourse.bacc as bacc
    import concourse.mybir as mybir
    import concourse.tile as tile

    f32 = mybir.dt.float32
    bf16 = mybir.dt.bfloat16
    fp8 = mybir.dt.float8e4
    AFT = mybir.ActivationFunctionType
    ALU = mybir.AluOpType
    DR = mybir.MatmulPerfMode.DoubleRow

    nc = bacc.Bacc("TRN2", target_bir_lowering=False, debug=False,
                   num_devices=N_CORES)

    # host-packed inputs (see _host_prep for layouts)
    xc = nc.dram_tensor("xc", [IN_FLT, 16384], fp8, kind="ExternalInput")
    tsh = nc.dram_tensor("tsh", [NG * KP * 128, 4096], fp8, kind="ExternalInput")
    # conv weights padded to 16 cols per (pair, t) slot: DoubleRow LDWEIGHTS
    # requires the pair-dim stride to be a multiple of 16 elements.
    wc = nc.dram_tensor("wc", [IN_FLT, 256], fp8, kind="ExternalInput")
    wd = nc.dram_tensor("wd", [OC, 512], bf16, kind="ExternalInput")
    eye = nc.dram_tensor("eye", [B, B], f32, kind="ExternalInput")
    # sgn zero-padded to 128 contraction rows: full PE row activity keeps the
    # HAM clock gate at K=8 (2.4 GHz) without changing matmul cycle count.
    sgn = nc.dram_tensor("sgn", [128, 512], bf16, kind="ExternalInput")
    inc = nc.dram_tensor("inc", [128, 128], bf16, kind="ExternalInput")
    y = nc.dram_tensor("y", [B, OC, 8, N], bf16, kind="ExternalOutput")

    with tile.TileContext(nc) as tc:
        with tc.tile_pool(name="const", bufs=1) as constp, \
             tc.tile_pool(name="dram", bufs=1, space="DRAM") as dram, \
             tc.tile_pool(name="xb", bufs=1) as xbp, \
             tc.tile_pool(name="Tp", bufs=24) as Tp, \
             tc.tile_pool(name="work", bufs=2) as wp, \
             tc.tile_pool(name="persist", bufs=1) as pp, \
             tc.tile_pool(name="psb", bufs=2, space="PSUM") as psb, \
             tc.tile_pool(name="ps_m", bufs=2, space="PSUM") as ps_m, \
             tc.tile_pool(name="ps_acc", bufs=1, space="PSUM") as ps_acc, \
             tc.tile_pool(name="ps_junk", bufs=1, space="PSUM") as ps_junk:

            # Dummy pre-sync collective, issued first with ZERO dependencies
            # (contents irrelevant): the first collective on a core pays a
            # ~37us CC-engine startup latency; burn it here, overlapped with
            # the x load / conv / T prefetch, so the real AllGather is fast.
            pre_in = dram.tile([1, 8], f32)
            pre_out = dram.tile([N_CORES, 8], f32)
            nc.gpsimd.collective_compute(
                "AllGather", ALU.bypass,
                replica_groups=[list(range(N_CORES))],
                ins=[pre_in.opt()], outs=[pre_out.opt()])

            wc_sb = constp.tile([IN_FLT, 256], fp8)
            nc.scalar.dma_start(wc_sb[:], wc[:])
            wd_sb = constp.tile([OC, 512], bf16)
            nc.scalar.dma_start(wd_sb[:], wd[:])
            eye_sb = constp.tile([B, B], f32)
            nc.scalar.dma_start(eye_sb[:], eye[:])
            sgn_sb = constp.tile([128, 512], bf16)
            nc.scalar.dma_start(sgn_sb[:], sgn[:])
            inc_sb = constp.tile([128, 128], bf16)
            nc.scalar.dma_start(inc_sb[:], inc[:])

            # junk-matmul helpers: keep the PE HAM activity monitor above its
            # warm threshold so the clock gate stays at 2.4 GHz. Outputs go to
            # a dedicated PSUM bank that is never read.
            jpsum = ps_junk.tile([128, 512], f32, tag="junk")
            xb_bf = None

            def _junk(n=1):
                for _ in range(n):
                    nc.tensor.matmul(jpsum[:], inc_sb[:],
                                     xb_bf[:, :512], start=True, stop=True)



            # ---- Stage A: conv (fp8 DoubleRow over (c, rs-pair) contraction)
            xrl = pp.tile([3, BC * 256], f32)        # col = b*256 + i*16 + j
            xball = xbp.tile([IN_FLT, 16384], fp8, tag="xb")
            nc.sync.dma_start(xball[:], xc[:])
            xb_bf = xball[:].bitcast(bf16)
            xc_r = xball[:].rearrange("c (k t n) -> c k t n", k=8, t=2)
            wc_r = wc_sb[:].rearrange("c (k t o) -> c k t o", k=8, t=2)
            for half in range(2):
                psc_t = psb.tile([128, 1024], f32, tag="big")
                psc = psc_t[:3, :512]
                for kp in range(8):
                    nc.tensor.matmul(
                        psc, wc_r[:, kp, :, :3],
                        xc_r[:, kp, :, half * 512:(half + 1) * 512],
                        start=(kp == 0), stop=(kp == 7), perf_mode=DR)
                nc.vector.tensor_copy(xrl[:, half * 512:(half + 1) * 512], psc)

            # Mb double-buffer: persistent 128-partition tiles whose rows
            # 32..127 stay zero forever (psD contraction zero-padding).
            Mbs = [pp.tile([128, 2048], bf16, tag=f"Mb{i}", name=f"Mb{i}")
                   for i in range(2)]
            for mb in Mbs:
                for q in range(1, 4):
                    nc.gpsimd.memset(mb[q * 32:(q + 1) * 32, :], 0.0)

            ag_in = dram.tile([BC, D_IN], f32)
            ag_out = dram.tile([B, D_IN], f32)
            nc.gpsimd.dma_start(
                ag_in[:].rearrange("b (c ij) -> c b ij", c=3),
                xrl[:].rearrange("c (b ij) -> c b ij", b=BC))
            nc.gpsimd.collective_compute(
                "AllGather", ALU.bypass,
                replica_groups=[list(range(N_CORES))],
                ins=[ag_in.opt()], outs=[ag_out.opt()])
            _junk(64)   # keep PE busy (and HAM-warm) through the AllGather window

            # ---- Stage B: x_r^T (128 d_in x 32 b per chunk), cast to fp8 x2
            xr_all = pp.tile([B, D_IN], f32)
            nc.scalar.dma_start(xr_all[:], ag_out[:])
            xrT = pp.tile([128, 6 * B], fp8)
            for k in range(6):
                pst_t = psb.tile([128, 1024], f32, tag="big")
                pst = pst_t[:, :B]
                nc.tensor.transpose(pst[:], xr_all[:, k * 128:(k + 1) * 128], eye_sb[:])
                nc.scalar.mul(xrT[:, k * B:(k + 1) * B], pst[:], XR_SCALE)
                _junk(2)
            xrT_r = xrT[:].rearrange("p (c two b) -> p c two b", c=KP, two=2)

            acc = pp.tile([128, NG * B], f32)        # col = g*32 + b
            acc2 = pp.tile([OC, 32 * B], bf16)       # (32 ch, col = rj*32 + b)
            wd_v = wd_sb[:].rearrange("c (m v) -> c v m", v=4)

            def _deconv_r(r):
                # acc cols for g in [4r, 4r+4) -> acc2 (32 ch, (rj, b)) band
                acc2_3 = acc2[:].rearrange("c (g x b) -> c g x b", g=NG, x=4)
                for q in range(4):
                    nc.gpsimd.dma_start(
                        acc2_3[:, 4 * r:4 * r + 4, q, :],
                        acc[q * 32:(q + 1) * 32, 4 * r * B:(4 * r + 4) * B]
                        .rearrange("c (g b) -> c g b", g=4))
                yst = wp.tile([128, B * N], bf16, tag="yst")  # col = b*64 + 4j + v
                yst_r = yst[:].rearrange("p (b j v) -> p j b v", j=16, v=4)
                for v in range(4):
                    psdt = ps_junk.tile([128, 512], f32, tag="junk",
                                        name=f"psd_{r}_{v}")
                    nc.tensor.matmul(
                        psdt[:], wd_v[:, v], acc2[:, r * 512:(r + 1) * 512],
                        start=True, stop=True)
                    nc.scalar.copy(
                        yst_r[:, :, :, v],
                        psdt[:].rearrange("p (j b q) -> p j b q", j=16, q=1))
                for u in range(4):
                    nc.sync.dma_start(
                        y[:, :, 4 * r + u, :].rearrange("b o c -> o b c"),
                        yst[u * 32:(u + 1) * 32, :].rearrange("o (b c) -> o b c", c=N))

            # ---- Stages C/D fused per dgroup g
            # M_b = x_r @ T_g : (32 b, 2048 = (s128, f16))  [T streamed as rhs, DR]
            # D = sgn^T @ M_b : (128 pairs, (s, f)) in PSUM
            # dist = reduce_|.|_f(D) ; E = exp(-dist) bf16
            # acc_g = E^T @ inc : (128 s, 32 b) accumulated over pair chunks
            for g in range(NG):
                Ts = []
                for kp in range(KP):
                    Tt = Tp.tile([128, 4096], fp8, tag="T")
                    row = (g * KP + kp) * 128
                    nc.sync.dma_start(Tt[:], tsh[row:row + 128, :])
                    Ts.append(Tt)
                Mb = Mbs[g % 2]                           # (128, (s, f)); rows 32+ zero
                for ncn in range(4):
                    psm = ps_m.tile([B, 512], f32, tag="mm")
                    for kp in range(KP):
                        nc.tensor.matmul(
                            psm[:], xrT_r[:, kp],
                            Ts[kp][:].rearrange("p (two n) -> p two n", two=2)
                            [:, :, ncn * 512:(ncn + 1) * 512],
                            start=(kp == 0), stop=(kp == KP - 1), perf_mode=DR)
                    nc.scalar.mul(Mb[:B, ncn * 512:(ncn + 1) * 512], psm[:], M_DESCALE)
                accg = ps_acc.tile([128, B], f32, tag="accg")
                for pc in range(4):
                    dist = wp.tile([128, 128], f32, tag="dist")
                    for nh in range(2):
                        psD_t = psb.tile([128, 1024], f32, tag="big")
                        for nq in range(2):
                            ncn = nh * 2 + nq
                            nc.tensor.matmul(
                                psD_t[:, nq * 512:(nq + 1) * 512],
                                sgn_sb[:, pc * 128:(pc + 1) * 128],
                                Mb[:, ncn * 512:(ncn + 1) * 512],
                                start=True, stop=True)
                        nc.vector.tensor_reduce(
                            dist[:, nh * 64:(nh + 1) * 64],
                            psD_t[:].rearrange("p (s f) -> p s f", f=F),
                            axis=mybir.AxisListType.X, op=ALU.add,
                            apply_absolute_value=True)
                    Egp = wp.tile([128, 128], bf16, tag="E")
                    nc.scalar.activation(Egp[:], dist[:], AFT.Exp, scale=-1.0)
                    nc.tensor.matmul(
                        accg[:], Egp[:], inc_sb[:, pc * B:(pc + 1) * B],
                        start=(pc == 0), stop=(pc == 3))
                    _junk(2)
                nc.scalar.copy(acc[:, g * B:(g + 1) * B], accg[:])
                if g in (NG // 2 - 1, NG - 1):
                    _deconv_r(g // (NG // 2))

    nc.finalize()
    return nc


def _host_prep(x, w_conv, T, w_deconv):
    """Build the 8 per-core input maps."""
    bf = ml_dtypes.bfloat16
    f8 = ml_dtypes.float8_e4m3

    def e4(v):
        return np.clip(v, -240.0, 240.0).astype(f8)

    # x: per-core 4 samples, packed for fp8 DoubleRow conv:
    # xh[c, kp, t, b*256+i*16+j] = x[b, c, 4i+r, 4j+(2u+t)], kp = r*2+u
    # T: (768, 8192, 16) -> per-core (768, 32ch, 2i-rows, 16j, 16f)
    Tr = np.ascontiguousarray(T).reshape(D_IN, OC, T_SP, T_SP, F)
    # conv weights packed to match: wc[c, kp, t, o(pad 16)]
    wcp = np.transpose(w_conv, (1, 2, 3, 0)).reshape(IN_FLT, 8, 2, 3)
    wc_host = np.zeros((IN_FLT, 8, 2, 16), np.float32)
    wc_host[:, :, :, :3] = wcp
    wc_host = e4(wc_host.reshape(IN_FLT, 256))
    # deconv weights: lhsT[ic, (u*32+oc)*4+v] = w_deconv[oc, ic, u, v]
    wd_host = np.ascontiguousarray(
        np.transpose(w_deconv, (1, 2, 0, 3)).reshape(OC, 512)).astype(bf)
    eye_host = np.eye(B, dtype=np.float32)

    # pairwise sign matrix (b1 < b2, 496 pairs padded to 512) and incidence
    # (zero-padded to 128 contraction rows for full PE activity)
    pairs = [(a, b) for a in range(B) for b in range(a + 1, B)]
    sgn_host = np.zeros((128, 512), np.float32)
    inc_host = np.zeros((128, 128), np.float32)
    for p, (a, b) in enumerate(pairs):
        sgn_host[a, p] = 1.0
        sgn_host[b, p] = -1.0
        inc_host[p % 128, (p // 128) * B + a] = 1.0
        inc_host[p % 128, (p // 128) * B + b] = 1.0
    sgn_host = sgn_host.astype(bf)
    inc_host = inc_host.astype(bf)

    Tq = e4(T * T_SCALE).reshape(D_IN, OC, T_SP, T_SP, F)

    in_maps = []
    for c in range(N_CORES):
        # x pack: (4b, 128c, 64, 64) -> [c, (r,u), t, (b, i, j)]
        xs = x[BC * c:BC * (c + 1)].reshape(BC, IN_FLT, T_SP, K, T_SP, 2, 2)
        # dims: b, ch, i, r, j, u, t  ->  ch, r, u, t, b, i, j
        xh = np.ascontiguousarray(xs.transpose(1, 3, 5, 6, 0, 2, 4)).reshape(
            IN_FLT, 16384)
        # T shard: i rows 2c, 2c+1; column order s=(r*16+j)*32+ch, then f
        tslice = Tq[:, :, 2 * c:2 * c + 2, :, :]            # (768, ch, r, j, f)
        tshard = np.ascontiguousarray(
            np.transpose(tslice, (0, 2, 3, 1, 4)).reshape(D_IN, DSH * F))
        # DoubleRow pack: (3kp, 2two, 128p, cols per g 2048) -> rows (g,kp,p)
        t3 = tshard.reshape(KP, 2, 128, NG, 2048)
        tpk = np.ascontiguousarray(t3.transpose(3, 0, 2, 1, 4)).reshape(
            NG * KP * 128, 4096)
        in_maps.append({
            "xc": e4(xh),
            "tsh": tpk,
            "wc": wc_host,
            "wd": wd_host,
            "eye": eye_host,
            "sgn": sgn_host,
            "inc": inc_host,
        })
    return in_maps


def _get_nc():
    if "nc" not in _CACHE:
        _CACHE["nc"] = _build_nc()
    return _CACHE["nc"]


def run(inputs, trace=False, trace_kwargs=None):
    """Run on hardware; returns (full_output, BassKernelResults)."""
    from concourse.bass_utils import run_bass_kernel_spmd
    nc = _get_nc()
    in_maps = _host_prep(inputs["x"], inputs["w_conv"], inputs["T"],
                         inputs["w_deconv"])
    res = run_bass_kernel_spmd(nc, in_maps, list(range(N_CORES)), trace=trace,
                               **(trace_kwargs or {}))
    x = np.asarray(inputs["x"], dtype=np.float32)
    full = np.empty((B, IN_FLT + OC, N, N), np.float32)
    full[:, :IN_FLT] = x
    for c in range(N_CORES):
        full[:, IN_FLT:, 8 * c:8 * (c + 1), :] = res.results[c]["y"].astype(
            np.float32)
    return full, res


def kernel(**inputs) -> np.ndarray:
    out, _ = run(inputs, trace=False)
    return out


# revision 44
# speedup vs baseline: 1.1032x; 1.1032x over previous
"""Trainium2 Bass kernel for MinibatchDiscrimination2d.

Full computation:
  x (32,128,64,64) --conv s4--> x_r (32,3,16,16)
  M = x_r @ T  -> (32, 8192, 16)
  dist[b1,b2,d] = sum_f |M[b1,d,f]-M[b2,d,f]|
  out[b,d] = sum_b2 exp(-dist) - 1 -> (32,32,16,16)
  out_a = deconv s4 (32,32,64,64); return concat([x, out_a], ch)

Sharding over 8 cores: split the t*t=256 output spatial positions of the
D_OUT axis into 8 row-bands (2 of 16 t-rows per core). Each core gets a
(768, 1024, 16) slice of T (fp8 e4m3, x128 scale), computes M/dist/out
for its band for ALL 32 samples, and deconvs its band into 8 of the 64
output rows. The conv runs data-parallel over B (4 samples/core, fp8
DoubleRow) followed by an AllGather of the tiny x_r. x passes through on
the host (concat identity part).

fp8: T scaled x128, xr scaled x2 -> M psum = 256*M, descaled on the
PSUM->SBUF copy. M-matmul and conv use DoubleRow (2 contraction rows
per partition).

Per-core d index:  s = (r*16 + j)*32 + ch   (r in 0..1, j in 0..15, ch in 0..31)
dgroup g = s // 128; partition p = s % 128 = (rj%4)*32 + ch.
"""

import numpy as np
import ml_dtypes

N_CORES = 8
B, IN_FLT, N = 32, 128, 64
K = 4
T_SP = 16
OC = 32
F = 16
D_IN = 768
BC = B // N_CORES          # 4 samples per core (conv data-parallel)
DSH = 1024                 # d per core
NG = DSH // 128            # 8 dgroups
KP = 3                     # contraction pairs (768 = 3 * 256) for DoubleRow
T_SCALE = 128.0
XR_SCALE = 2.0
M_DESCALE = 1.0 / (T_SCALE * XR_SCALE)

_CACHE = {}


def _build_nc():
    import concourse.bacc as bacc
    import concourse.mybir as mybir
    import concourse.tile as tile

    f32 = mybir.dt.float32
    bf16 = mybir.dt.bfloat16
    fp8 = mybir.dt.float8e4
    AFT = mybir.ActivationFunctionType
    ALU = mybir.AluOpType
    DR = mybir.MatmulPerfMode.DoubleRow

    nc = bacc.Bacc("TRN2", target_bir_lowering=False, debug=False,
                   num_devices=N_CORES)

    # host-packed inputs (see _host_prep for layouts)
    xc = nc.dram_tensor("xc", [IN_FLT, 16384], fp8, kind="ExternalInput")
    tsh = nc.dram_tensor("tsh", [NG * KP * 128, 4096], fp8, kind="ExternalInput")
    # conv weights padded to 16 cols per (pair, t) slot: DoubleRow LDWEIGHTS
    # requires the pair-dim stride to be a multiple of 16 elements.
    wc = nc.dram_tensor("wc", [IN_FLT, 256], fp8, kind="ExternalInput")
    # wd as 16 zero-padded (128,128) blocks [q, v]: block (q,v) is nonzero
    # only on partitions [32q, 32q+32), so the deconv can contract acc's
    # q-th partition block using a full-128-partition matmul.
    wd = nc.dram_tensor("wd", [128, 2048], f32, kind="ExternalInput")
    eye = nc.dram_tensor("eye", [B, B], f32, kind="ExternalInput")
    # sgn zero-padded to 128 contraction rows: full PE row activity keeps the
    # HAM clock gate at K=8 (2.4 GHz) without changing matmul cycle count.
    sgn = nc.dram_tensor("sgn", [128, 512], bf16, kind="ExternalInput")
    inc = nc.dram_tensor("inc", [128, 128], bf16, kind="ExternalInput")
    # y layout [v, row(4r+u), oc, j, b]: makes deconv PSUM->SBUF copies and
    # the final stores fully contiguous; host untangles the interleave.
    y = nc.dram_tensor("y", [4, 8, OC, T_SP, B], bf16, kind="ExternalOutput")

    with tile.TileContext(nc) as tc:
        with tc.tile_pool(name="const", bufs=1) as constp, \
             tc.tile_pool(name="dram", bufs=1, space="DRAM") as dram, \
             tc.tile_pool(name="xb", bufs=1) as xbp, \
             tc.tile_pool(name="Tp", bufs=24) as Tp, \
             tc.tile_pool(name="work", bufs=2) as wp, \
             tc.tile_pool(name="persist", bufs=1) as pp, \
             tc.tile_pool(name="psb", bufs=2, space="PSUM") as psb, \
             tc.tile_pool(name="ps_m", bufs=2, space="PSUM") as ps_m, \
             tc.tile_pool(name="ps_acc", bufs=1, space="PSUM") as ps_acc, \
             tc.tile_pool(name="ps_junk", bufs=1, space="PSUM") as ps_junk:

            wc_sb = constp.tile([IN_FLT, 256], fp8)
            nc.scalar.dma_start(wc_sb[:], wc[:])
            wd_sb = constp.tile([128, 2048], f32)
            nc.scalar.dma_start(wd_sb[:], wd[:])
            eye_sb = constp.tile([B, B], f32)
            nc.scalar.dma_start(eye_sb[:], eye[:])
            sgn_sb = constp.tile([128, 512], bf16)
            nc.scalar.dma_start(sgn_sb[:], sgn[:])
            inc_sb = constp.tile([128, 128], bf16)
            nc.scalar.dma_start(inc_sb[:], inc[:])

            # junk-matmul helpers: keep the PE HAM activity monitor above its
            # warm threshold so the clock gate stays at 2.4 GHz. Outputs go to
            # a dedicated PSUM bank that is never read.
            jpsum = ps_junk.tile([128, 512], f32, tag="junk")
            xb_bf = None

            def _junk(n=1):
                for _ in range(n):
                    nc.tensor.matmul(jpsum[:], inc_sb[:],
                                     xb_bf[:, :512], start=True, stop=True)



            # ---- Stage A: conv (fp8 DoubleRow over (c, rs-pair) contraction)
            xrl = pp.tile([3, BC * 256], f32)        # col = b*256 + i*16 + j
            xball = xbp.tile([IN_FLT, 16384], fp8, tag="xb")
            nc.sync.dma_start(xball[:], xc[:])
            xb_bf = xball[:].bitcast(bf16)
            xc_r = xball[:].rearrange("c (k t n) -> c k t n", k=8, t=2)
            wc_r = wc_sb[:].rearrange("c (k t o) -> c k t o", k=8, t=2)
            for half in range(2):
                psc_t = psb.tile([128, 1024], f32, tag="big")
                psc = psc_t[:3, :512]
                for kp in range(8):
                    nc.tensor.matmul(
                        psc, wc_r[:, kp, :, :3],
                        xc_r[:, kp, :, half * 512:(half + 1) * 512],
                        start=(kp == 0), stop=(kp == 7), perf_mode=DR)
                nc.vector.tensor_copy(xrl[:, half * 512:(half + 1) * 512], psc)

            ag_in = dram.tile([BC, D_IN], f32)
            ag_out = dram.tile([B, D_IN], f32)
            nc.gpsimd.dma_start(
                ag_in[:].rearrange("b (c ij) -> c b ij", c=3),
                xrl[:].rearrange("c (b ij) -> c b ij", b=BC))
            nc.gpsimd.collective_compute(
                "AllGather", ALU.bypass,
                replica_groups=[list(range(N_CORES))],
                ins=[ag_in.opt()], outs=[ag_out.opt()])

            # Mb double-buffer: persistent 128-partition tiles whose rows
            # 32..127 stay zero forever (psD contraction zero-padding).
            # Memsets traced after the collective so they don't delay its
            # staging on the gpsimd queue.
            Mbs = [pp.tile([128, 2048], bf16, tag=f"Mb{i}", name=f"Mb{i}")
                   for i in range(2)]
            for mb in Mbs:
                for q in range(1, 4):
                    nc.gpsimd.memset(mb[q * 32:(q + 1) * 32, :], 0.0)
            _junk(64)   # keep PE busy (and HAM-warm) through the AllGather window

            # ---- Stage B: x_r^T (128 d_in x 32 b per chunk), cast to fp8 x2
            xr_all = pp.tile([B, D_IN], f32)
            nc.scalar.dma_start(xr_all[:], ag_out[:])
            xrT = pp.tile([128, 6 * B], fp8)
            for k in range(6):
                pst_t = psb.tile([128, 1024], f32, tag="big")
                pst = pst_t[:, :B]
                nc.tensor.transpose(pst[:], xr_all[:, k * 128:(k + 1) * 128], eye_sb[:])
                nc.scalar.mul(xrT[:, k * B:(k + 1) * B], pst[:], XR_SCALE)
                _junk(2)
            xrT_r = xrT[:].rearrange("p (c two b) -> p c two b", c=KP, two=2)

            acc = pp.tile([128, NG * B], f32)        # col = g*32 + b
            wd_v = wd_sb[:].rearrange("p (q v m) -> p q v m", q=4, v=4)

            def _deconv_r(r):
                # deconv reads acc directly (f32 x f32 matmul, no staging):
                # out[(u,oc), j*32+b] += wd[:,v,(u,oc)]^T @ acc[q-rows, (g,b)]
                # with j = g*4 + q; one matmul per (v, q).
                yst = wp.tile([128, 2048], bf16, tag="yst")  # cols (v, j, b)
                for v in range(4):
                    psdt = ps_junk.tile([128, 512], f32, tag="junk",
                                        name=f"psd_{r}_{v}")
                    psd_q = psdt[:].rearrange("p (g q b) -> p g q b", g=4, q=4)
                    for q in range(4):
                        nc.tensor.matmul(
                            psd_q[:, :, q, :],
                            wd_v[:, q, v],
                            acc[:, 4 * r * B:(4 * r + 4) * B],
                            start=True, stop=True)
                    nc.scalar.copy(yst[:, v * 512:(v + 1) * 512], psdt[:])
                for u in range(4):
                    for v in range(4):
                        nc.sync.dma_start(
                            y[v, 4 * r + u],
                            yst[u * 32:(u + 1) * 32, v * 512:(v + 1) * 512]
                            .rearrange("o (j b) -> o j b", j=T_SP))

            # ---- Stages C/D fused per dgroup g
            # M_b = x_r @ T_g : (32 b, 2048 = (s128, f16))  [T streamed as rhs, DR]
            # D = sgn^T @ M_b : (128 pairs, (s, f)) in PSUM
            # dist = reduce_|.|_f(D) ; E = exp(-dist) bf16
            # acc_g = E^T @ inc : (128 s, 32 b) accumulated over pair chunks
            for g in range(NG):
                Ts = []
                for kp in range(KP):
                    Tt = Tp.tile([128, 4096], fp8, tag="T")
                    row = (g * KP + kp) * 128
                    # issued from the scalar queue: keeps the sync HWDGE ring
                    # quiet so the collective isn't starved behind the stream
                    nc.scalar.dma_start(Tt[:], tsh[row:row + 128, :])
                    Ts.append(Tt)
                Mb = Mbs[g % 2]                           # (128, (s, f)); rows 32+ zero
                for ncn in range(4):
                    psm = ps_m.tile([B, 512], f32, tag="mm")
                    for kp in range(KP):
                        nc.tensor.matmul(
                            psm[:], xrT_r[:, kp],
                            Ts[kp][:].rearrange("p (two n) -> p two n", two=2)
                            [:, :, ncn * 512:(ncn + 1) * 512],
                            start=(kp == 0), stop=(kp == KP - 1), perf_mode=DR)
                    nc.scalar.mul(Mb[:B, ncn * 512:(ncn + 1) * 512], psm[:], M_DESCALE)
                accg = ps_acc.tile([128, B], f32, tag="accg")
                for pc in range(4):
                    dist = wp.tile([128, 128], f32, tag="dist")
                    for nh in range(2):
                        psD_t = psb.tile([128, 1024], f32, tag="big")
                        for nq in range(2):
                            ncn = nh * 2 + nq
                            nc.tensor.matmul(
                                psD_t[:, nq * 512:(nq + 1) * 512],
                                sgn_sb[:, pc * 128:(pc + 1) * 128],
                                Mb[:, ncn * 512:(ncn + 1) * 512],
                                start=True, stop=True)
                        nc.vector.tensor_reduce(
                            dist[:, nh * 64:(nh + 1) * 64],
                            psD_t[:].rearrange("p (s f) -> p s f", f=F),
                            axis=mybir.AxisListType.X, op=ALU.add,
                            apply_absolute_value=True)
                    Egp = wp.tile([128, 128], bf16, tag="E")
                    nc.scalar.activation(Egp[:], dist[:], AFT.Exp, scale=-1.0)
                    nc.tensor.matmul(
                        accg[:], Egp[:], inc_sb[:, pc * B:(pc + 1) * B],
                        start=(pc == 0), stop=(pc == 3))
                    _junk(2)
                nc.scalar.copy(acc[:, g * B:(g + 1) * B], accg[:])
                if g in (NG // 2 - 1, NG - 1):
                    _deconv_r(g // (NG // 2))

    nc.finalize()
    return nc


def _host_prep(x, w_conv, T, w_deconv):
    """Build the 8 per-core input maps."""
    bf = ml_dtypes.bfloat16
    f8 = ml_dtypes.float8_e4m3

    def e4(v):
        return np.clip(v, -240.0, 240.0).astype(f8)

    # x: per-core 4 samples, packed for fp8 DoubleRow conv:
    # xh[c, kp, t, b*256+i*16+j] = x[b, c, 4i+r, 4j+(2u+t)], kp = r*2+u
    # T: (768, 8192, 16) -> per-core (768, 32ch, 2i-rows, 16j, 16f)
    Tr = np.ascontiguousarray(T).reshape(D_IN, OC, T_SP, T_SP, F)
    # conv weights packed to match: wc[c, kp, t, o(pad 16)]
    wcp = np.transpose(w_conv, (1, 2, 3, 0)).reshape(IN_FLT, 8, 2, 3)
    wc_host = np.zeros((IN_FLT, 8, 2, 16), np.float32)
    wc_host[:, :, :, :3] = wcp
    wc_host = e4(wc_host.reshape(IN_FLT, 256))
    # deconv weights: block (q, v) nonzero only on partitions [32q, 32q+32),
    # wd_host[32q+ic, q, v, u*32+oc] = w_deconv[oc, ic, u, v]
    wd0 = np.transpose(w_deconv, (1, 3, 2, 0)).reshape(OC, 4, 128)  # [c, v, m]
    wdq = np.zeros((128, 4, 4, 128), np.float32)
    for q in range(4):
        wdq[32 * q:32 * (q + 1), q] = wd0
    wd_host = wdq.reshape(128, 2048)
    eye_host = np.eye(B, dtype=np.float32)

    # pairwise sign matrix (b1 < b2, 496 pairs padded to 512) and incidence
    # (zero-padded to 128 contraction rows for full PE activity)
    pairs = [(a, b) for a in range(B) for b in range(a + 1, B)]
    sgn_host = np.zeros((128, 512), np.float32)
    inc_host = np.zeros((128, 128), np.float32)
    for p, (a, b) in enumerate(pairs):
        sgn_host[a, p] = 1.0
        sgn_host[b, p] = -1.0
        inc_host[p % 128, (p // 128) * B + a] = 1.0
        inc_host[p % 128, (p // 128) * B + b] = 1.0
    sgn_host = sgn_host.astype(bf)
    inc_host = inc_host.astype(bf)

    Tq = e4(T * T_SCALE).reshape(D_IN, OC, T_SP, T_SP, F)

    in_maps = []
    for c in range(N_CORES):
        # x pack: (4b, 128c, 64, 64) -> [c, (r,u), t, (b, i, j)]
        xs = x[BC * c:BC * (c + 1)].reshape(BC, IN_FLT, T_SP, K, T_SP, 2, 2)
        # dims: b, ch, i, r, j, u, t  ->  ch, r, u, t, b, i, j
        xh = np.ascontiguousarray(xs.transpose(1, 3, 5, 6, 0, 2, 4)).reshape(
            IN_FLT, 16384)
        # T shard: i rows 2c, 2c+1; column order s=(r*16+j)*32+ch, then f
        tslice = Tq[:, :, 2 * c:2 * c + 2, :, :]            # (768, ch, r, j, f)
        tshard = np.ascontiguousarray(
            np.transpose(tslice, (0, 2, 3, 1, 4)).reshape(D_IN, DSH * F))
        # DoubleRow pack: (3kp, 2two, 128p, cols per g 2048) -> rows (g,kp,p)
        t3 = tshard.reshape(KP, 2, 128, NG, 2048)
        tpk = np.ascontiguousarray(t3.transpose(3, 0, 2, 1, 4)).reshape(
            NG * KP * 128, 4096)
        in_maps.append({
            "xc": e4(xh),
            "tsh": tpk,
            "wc": wc_host,
            "wd": wd_host,
            "eye": eye_host,
            "sgn": sgn_host,
            "inc": inc_host,
        })
    return in_maps


def _get_nc():
    if "nc" not in _CACHE:
        _CACHE["nc"] = _build_nc()
    return _CACHE["nc"]


def run(inputs, trace=False, trace_kwargs=None):
    """Run on hardware; returns (full_output, BassKernelResults)."""
    from concourse.bass_utils import run_bass_kernel_spmd
    nc = _get_nc()
    in_maps = _host_prep(inputs["x"], inputs["w_conv"], inputs["T"],
                         inputs["w_deconv"])
    res = run_bass_kernel_spmd(nc, in_maps, list(range(N_CORES)), trace=trace,
                               **(trace_kwargs or {}))
    x = np.asarray(inputs["x"], dtype=np.float32)
    full = np.empty((B, IN_FLT + OC, N, N), np.float32)
    full[:, :IN_FLT] = x
    for c in range(N_CORES):
        ya = res.results[c]["y"].astype(np.float32)   # [v, row, oc, j, b]
        # full[b, 128+oc, 8c+row, 4j+v] = ya[v, row, oc, j, b]
        yh = np.transpose(ya, (4, 2, 1, 3, 0)).reshape(B, OC, 8, N)
        full[:, IN_FLT:, 8 * c:8 * (c + 1), :] = yh
    return full, res


def kernel(**inputs) -> np.ndarray:
    out, _ = run(inputs, trace=False)
    return out
